# revision 29
# baseline (speedup 1.0000x reference)
"""AUCM loss kernel for Trainium2 (8 NeuronCores, raw Bass).

Reference math (N = 16384 preds, int32 targets):
    pos = preds[targets==1]; neg = preds[targets==0]
    d_ij = 1 - (pos_i - neg_j)
    loss = mean_ij [ d_ij^2 + MARGIN*relu(d_ij) ]

Decomposition: with u_i = 1 - pos_i and v_j = neg_j, d_ij = u_i + v_j.
    sum_ij d^2     = Nv*sum(u^2) + 2*sum(u)*sum(v) + Nu*sum(v^2)  (host, O(N))
    sum_ij relu(d) = the real O(Nu*Nv) work -> computed on device.

Device strategy (no TensorEngine; ScalarE and VectorE both stream the whole
pairwise grid directly out of SBUF — a PE-built D matrix would cap both
consumers at the PE's own column rate):
  - One side of the pair grid ("bias side") is laid out as 128-row blocks,
    one column [128,1] per block; the other side ("stream side") is
    DMA-broadcast to all 128 partitions: stream [128, S] fp32.
  - ScalarE, one instruction per (block, chunk):
        ACTIVATE(Relu, in=stream, bias=bias_col, accum_out) ->
        per-partition sum_j relu(stream_j + bias_p); -1e30 padding on
        either side makes relu() return 0 for any padded pair.
  - VectorE, one instruction per (block, chunk), using
        relu(s + b) = b + max(s, -b):
        TENSOR_SCALAR(max, scalar1=-bias_col, reduce=add, accum_out) ->
        per-partition sum_j max(stream_j, -bias_p). The host adds the
        fd*bias_p correction in float64 (padded stream elements contribute
        max(-1e30,-b) = -b, cancelled exactly by +b; padded bias rows are
        dropped on the host).
  - Each unit's accum lands in its own column of acc; the [128, na+nd]
    matrix is DMA'd out raw and the host does the final combine in f64.

The kernel is raw Bass (no TileContext): a 3-engine pipeline with three
semaphores (dma_in, act_done, dve_done). This avoids Tile's multi-
microsecond semaphore-init preamble and end-of-kernel barrier butterfly.
A dummy ACTIVATE hoists the ~1.5us ACT table load into the DMA window, and
the ACT/DVE unit split is load-balanced with one boundary unit split across
both engines.

Sharding: the longer of (pos, neg) is split evenly across the 8 cores, the
shorter is replicated. Whether the sharded side becomes the bias side or the
stream side is chosen per shape to minimize the modeled makespan (fewer,
fatter instructions win).
"""

import math
import os
import sys

import numpy as np

for _p in ("/opt/trn_rl_repo", "/root/.axon_site/_ro/trn_rl_repo"):
    if os.path.isdir(_p) and _p not in sys.path:
        sys.path.append(_p)

import concourse.bacc as bacc
import concourse.bass as bass
from concourse import mybir
from concourse.bass_utils import run_bass_kernel_spmd

N_CORES = 8
MARGIN = 1.0
NEG_BIG = -1.0e30
CHUNK = 4096  # max free-dim per consumer instruction

# test-harness hooks (the grading path never touches these)
TRACE = False
LAST_EXEC_NS = None
LAST_RESULTS = None

_prog_cache: dict = {}


def _act_cost(fd):
    return (250.0 + fd) / 1.2 + 181.0  # ACTIVATE + READ_ACCUMULATOR (measured)


def _dve_cost(fd):
    return (130.0 + fd) / 0.96  # measured


def _units(bp, s):
    """(block, c0, fd) units; guarantees at least two units total."""
    chunks = []
    c0 = 0
    while c0 < s:
        fd = min(CHUNK, s - c0)
        chunks.append((c0, fd))
        c0 += fd
    units = [(b, c0, fd) for b in range(bp) for c0, fd in chunks]
    if len(units) == 1:
        b, c0, fd = units[0]
        h = max(2, fd // 2) // 2 * 2  # even split
        units = [(b, c0, h), (b, c0 + h, fd - h)]
    return units


def _assign(raw_units):
    """Greedy ACT/DVE split by measured cost, then split one boundary unit
    to equalize finish times (ACT biased to finish first).

    Returns list of (b, c0, fd, eng, idx) work items plus (na, nd).
    """
    load_a = load_d = 0.0
    tagged = []
    for b, c0, fd in raw_units:
        if load_a + _act_cost(fd) <= load_d + _dve_cost(fd):
            tagged.append([b, c0, fd, "A"])
            load_a += _act_cost(fd)
        else:
            tagged.append([b, c0, fd, "D"])
            load_d += _dve_cost(fd)
    if abs(load_a - load_d) > 400.0:
        heavy = "A" if load_a > load_d else "D"
        k = max(i for i, t in enumerate(tagged) if t[3] == heavy)
        b, c0, fd, _ = tagged[k]
        if fd >= 256:
            base_a = load_a - (_act_cost(fd) if heavy == "A" else 0.0)
            base_d = load_d - (_dve_cost(fd) if heavy == "D" else 0.0)
            best = None
            for x in range(128, fd - 127, 64):  # x cols stay on ACT
                fa = base_a + _act_cost(x) + 200.0  # bias: ACT finishes first
                fdv = base_d + _dve_cost(fd - x)
                m = max(fa, fdv)
                if best is None or m < best[0]:
                    best = (m, x)
            if best is not None and best[0] < max(load_a, load_d) - 200.0:
                x = best[1]
                tagged[k : k + 1] = [[b, c0, x, "A"], [b, c0 + x, fd - x, "D"]]
    na = nd = 0
    out = []
    for b, c0, fd, eng in tagged:
        if eng == "A":
            out.append((b, c0, fd, "A", na))
            na += 1
        else:
            out.append((b, c0, fd, "D", nd))
            nd += 1
    return out, na, nd


def _makespan(raw_units):
    work, _, _ = _assign(raw_units)
    la = sum(_act_cost(fd) for _b, _c, fd, eng, _i in work if eng == "A")
    ld = sum(_dve_cost(fd) for _b, _c, fd, eng, _i in work if eng == "D")
    return max(la, ld)


def _build(bp, s):
    """Raw Bass program for one core: bp 128-wide bias blocks x s stream."""
    key = (bp, s)
    if key in _prog_cache:
        return _prog_cache[key]

    f32 = mybir.dt.float32
    bf16 = mybir.dt.bfloat16
    work, na, nd = _assign(_units(bp, s))
    assert na >= 1 and nd >= 1

    nc = bacc.Bacc(None, target_bir_lowering=False)
    stream_t = nc.dram_tensor("stream", [s], f32, kind="ExternalInput")
    uu_t = nc.dram_tensor("uu", [128, 2 * bp], f32, kind="ExternalInput")
    out_t = nc.dram_tensor("out", [128, na + nd], f32, kind="ExternalOutput")

    # input DMA plan: stream stripes split over the two HWDGE sequencers
    nstripes = max(1, min(2, s // 128))
    sw = (s // nstripes + 127) // 128 * 128
    stripes = []
    c0 = 0
    while c0 < s:
        stripes.append((c0, min(sw, s - c0)))
        c0 += sw
    n_in = len(stripes) + 1

    # Each unit gets a private scratch slice (the engines' main outputs are
    # dead stores — only accum_out matters — but same-engine WAW reuse is
    # unsafe on deep pipelines and trips the race detector).
    offs = []
    scr_w = 1  # slot 0 reserved for the table-load dummy
    for _b, _c0, fd, _eng, _idx in work:
        offs.append(scr_w)
        scr_w += fd

    with (
        nc.sbuf_tensor([128, s], f32) as stream_sb,
        nc.sbuf_tensor([128, 2 * bp], f32) as uu_sb,
        nc.sbuf_tensor([128, na + nd], f32) as acc,
        nc.sbuf_tensor([128, scr_w], bf16) as scr,
        nc.semaphore("dma_in") as dma_in,
        nc.semaphore("act_done") as act_done,
        nc.semaphore("dve_done") as dve_done,
        nc.Block() as block,
    ):
        bias_sb = uu_sb[:, :bp]
        nbias_sb = uu_sb[:, bp:]
        h = stream_t[:]

        def stripe_dma(eng, st):
            c0, w = stripes[st]
            bc = bass.AP(tensor=h.tensor, offset=h.offset + c0, ap=[[0, 128], [1, w]])
            eng.dma_start(out=stream_sb[:, c0 : c0 + w], in_=bc).then_inc(dma_in, 16)

        issuers = {}
        for st in range(len(stripes)):
            issuers.setdefault(["scalar", "sync"][st % 2], []).append(st)

        def wait_inputs(eng):
            eng.wait_ge(dma_in, 16 * n_in)

        @block.sync
        def _(sync: bass.BassEngine):
            for st in issuers.get("sync", []):
                stripe_dma(sync, st)
            with nc.allow_non_contiguous_dma(reason="tiny [128, 2*bp] bias tile"):
                sync.dma_start(out=uu_sb[:, :], in_=uu_t[:, :]).then_inc(dma_in, 16)
            # stream results out as each consumer finishes (ACT is biased to
            # finish first)
            sync.wait_ge(act_done, 1)
            with nc.allow_non_contiguous_dma(reason="small accum outputs"):
                sync.dma_start(out=out_t[:, :na], in_=acc[:, :na]).then_inc(dma_in, 16)
            sync.wait_ge(dve_done, 1)
            with nc.allow_non_contiguous_dma(reason="small accum outputs"):
                sync.dma_start(out=out_t[:, na:], in_=acc[:, na:]).then_inc(dma_in, 16)
            sync.wait_ge(dma_in, 16 * (n_in + 2))

        @block.scalar
        def _(scalar: bass.BassEngine):
            # dummy activation: hoists the ~1.5us ACT_TABLE_LOAD before the
            # DMA wait so it overlaps the input transfer
            zero = nc.const_aps.scalar_like(0.0, scr[:, 0:1])
            scalar.activation(scr[:, 0:1], zero, mybir.ActivationFunctionType.Relu)
            for st in issuers.get("scalar", []):
                stripe_dma(scalar, st)
            wait_inputs(scalar)
            seen = 0
            for k, (b, c0, fd, eng, idx) in enumerate(work):
                if eng != "A":
                    continue
                seen += 1
                ins = scalar.activation(
                    scr[:, offs[k] : offs[k] + fd],
                    stream_sb[:, c0 : c0 + fd],
                    mybir.ActivationFunctionType.Relu,
                    bias=bias_sb[:, b : b + 1],
                    accum_out=acc[:, idx : idx + 1],
                )
                if seen == na:
                    ins.then_inc(act_done, 1)

        @block.vector
        def _(vector: bass.BassEngine):
            wait_inputs(vector)
            seen = 0
            for k, (b, c0, fd, eng, idx) in enumerate(work):
                if eng != "D":
                    continue
                seen += 1
                ins = vector.tensor_scalar(
                    scr[:, offs[k] : offs[k] + fd],
                    stream_sb[:, c0 : c0 + fd],
                    nbias_sb[:, b : b + 1],
                    None,
                    op0=mybir.AluOpType.max,
                    op1=mybir.AluOpType.add,
                    accum_out=acc[:, na + idx : na + idx + 1],
                )
                if seen == nd:
                    ins.then_inc(dve_done, 1)

    nc.finalize()
    _prog_cache[key] = (nc, work, na, nd)
    return _prog_cache[key]


def kernel(preds: np.ndarray, targets: np.ndarray) -> np.ndarray:
    global LAST_EXEC_NS, LAST_RESULTS

    p = np.asarray(preds, dtype=np.float32).reshape(-1)
    t = np.asarray(targets).reshape(-1)

    u = (1.0 - p[t == 1]).astype(np.float32)  # positive side
    v = p[t == 0].astype(np.float32)  # negative side
    nu, nv = u.size, v.size

    # Shard the side that minimizes per-core pair count (the longer side).
    def pairs_cost(nl, nt):
        nblk = max(1, math.ceil(nl / (128 * N_CORES)))
        qt = max(128, 128 * math.ceil(nt / 128))
        return nblk * 128 * qt, nblk, qt

    c_u, nblk_u, qt_u = pairs_cost(nu, nv)
    c_v, nblk_v, qt_v = pairs_cost(nv, nu)
    if c_u <= c_v:
        lvals, tvals, nblk_l, qt = u, v, nblk_u, qt_u
    else:
        lvals, tvals, nblk_l, qt = v, u, nblk_v, qt_v
    nl_real, nt_real = lvals.size, tvals.size

    # Orientation: which side becomes the bias (partition) side.
    #  cfg1: bias = sharded side blocks (bp=nblk_l), stream = replicated (qt)
    #  cfg2: bias = replicated side (qt/128 blocks), stream = core's shard
    s_core = nblk_l * 128
    cfg1 = (nblk_l, qt)
    cfg2 = (qt // 128, s_core)
    use_cfg1 = _makespan(_units(*cfg1)) <= _makespan(_units(*cfg2))
    bp, s = cfg1 if use_cfg1 else cfg2

    ltot = nblk_l * 128 * N_CORES
    (nc, work, na, nd) = _build(bp, s)

    in_maps = []
    bias64_per_core = []
    realp_per_core = []
    if use_cfg1:
        stream_full = np.full(s, NEG_BIG, dtype=np.float32)
        stream_full[:nt_real] = tvals
        bias_all = np.full(ltot, NEG_BIG, dtype=np.float32)
        bias_all[:nl_real] = lvals
        nbias_all = np.zeros(ltot, dtype=np.float32)
        nbias_all[:nl_real] = -lvals
        for c in range(N_CORES):
            sl = slice(c * s_core, (c + 1) * s_core)
            uu = np.concatenate(
                [bias_all[sl].reshape(bp, 128).T, nbias_all[sl].reshape(bp, 128).T],
                axis=1,
            )
            in_maps.append({"stream": stream_full, "uu": np.ascontiguousarray(uu)})
            bias64_per_core.append(bias_all[sl].astype(np.float64).reshape(bp, 128).T)
            realp_per_core.append(
                np.clip(nl_real - (c * s_core + np.arange(bp) * 128), 0, 128)
            )
    else:
        bias_all = np.full(qt, NEG_BIG, dtype=np.float32)
        bias_all[:nt_real] = tvals
        nbias_all = np.zeros(qt, dtype=np.float32)
        nbias_all[:nt_real] = -tvals
        uu = np.ascontiguousarray(
            np.concatenate(
                [bias_all.reshape(bp, 128).T, nbias_all.reshape(bp, 128).T], axis=1
            )
        )
        b64 = bias_all.astype(np.float64).reshape(bp, 128).T
        nreal = np.clip(nt_real - np.arange(bp) * 128, 0, 128)
        lpad = np.full(ltot, NEG_BIG, dtype=np.float32)
        lpad[:nl_real] = lvals
        for c in range(N_CORES):
            in_maps.append(
                {
                    "stream": np.ascontiguousarray(lpad[c * s_core : (c + 1) * s_core]),
                    "uu": uu,
                }
            )
            bias64_per_core.append(b64)
            realp_per_core.append(nreal)

    br = run_bass_kernel_spmd(nc, in_maps, list(range(N_CORES)), trace=TRACE)
    results = br.results
    LAST_EXEC_NS = getattr(br, "exec_time_ns", None)
    LAST_RESULTS = br

    relu_sum = 0.0
    for c in range(N_CORES):
        o = np.asarray(results[c]["out"], dtype=np.float64)
        b64 = bias64_per_core[c]
        nreal = realp_per_core[c]
        for b, c0, fd, eng, idx in work:
            if eng == "A":
                # padded pairs contribute exactly 0
                relu_sum += o[:, idx].sum()
            else:
                nr = int(nreal[b])
                if nr > 0:
                    # sum over real bias rows of (acc + fd*bias_p); padded
                    # stream elements inside acc contribute -bias_p each,
                    # cancelled exactly by +fd*bias_p
                    relu_sum += o[:nr, na + idx].sum() + fd * b64[:nr, b].sum()

    u64 = u.astype(np.float64)
    v64 = v.astype(np.float64)
    sq_sum = (
        nv * (u64 * u64).sum() + 2.0 * u64.sum() * v64.sum() + nu * (v64 * v64).sum()
    )
    num_pairs = np.float64(nu) * np.float64(nv)
    with np.errstate(divide="ignore", invalid="ignore"):
        loss = np.float32((sq_sum + MARGIN * relu_sum) / num_pairs)
    return np.asarray(loss, dtype=np.float32)
